# revision 1
# baseline (speedup 1.0000x reference)
"""Trainium2 Bass kernel: fused multi-head self-attention block (CrossAttention module).

Sharding: 8 cores, each handles one (batch, query-slice) pair:
  core c -> batch b = c // 4, query rows q0 = (c % 4) * 1024 .. +1024.
Each core computes K/V projections for its full batch (replicated across the 4
cores sharing a batch), Q projection for its query slice, all 8 heads of
attention for its queries, and the output projection for its rows.
Host folds the per-channel gammas into the (pre-transposed) weights, transposes
x once, and concatenates the per-core outputs.

On-chip dataflow (per core, all fp32):
  - kT[ko, n]  = WkT.T @ xT   (key channels on partitions)  -- JIT per head-pair
  - qT[qo, n]  = WqT.T @ xTq                                -- JIT per head-pair
  - v[k, vo]   = xT.T @ WvT, stored interleaved with a ones column per head
                 ("vone" [128, 8*65]) so the attention rowsum comes free
  - scoresT tile [key 128, q 512] = kT_h.T @ qT_h, two heads packed as PE
    row-tiles (K=64 each) into one 2-bank PSUM tile
  - E = exp(SCALE * scoresT) via ScalarE, PSUM -> SBUF ([128, 1024] per instr)
  - rT[dv(+rowsum), q] += vone_h.T @ E, accumulated over 32 key chunks in PSUM
  - normalize: recip(rowsum) broadcast (GpSimd) and multiply (DVE) -> rTn
  - outT[do, q] = WoT.T @ rTn + bo
"""

import os
import sys

import numpy as np

for _p in ("/opt/trn_rl_repo", "/root/.axon_site/_ro/trn_rl_repo"):
    if os.path.isdir(_p) and _p not in sys.path:
        sys.path.append(_p)

B, N, D = 2, 4096, 512
H, DH = 8, 64
SCALE = DH ** -0.5
NCORES = 8
QPC = (B * N) // NCORES  # 1024 query rows per core
P = 128
CD = D // P              # 4 contraction chunks of 128
KC = N // P              # 32 key chunks of 128
NT = N // 512            # 8 key-column tiles of 512
QT = QPC // 512          # 2 query tiles of 512
HP = H // 2              # 4 head pairs

_PROGRAM = None
LAST_RESULT = None


def _build_program():
    import concourse.tile as tile
    from concourse import bacc, mybir

    f32 = mybir.dt.float32
    bf16 = mybir.dt.bfloat16
    AF = mybir.ActivationFunctionType
    OP = mybir.AluOpType

    nc = bacc.Bacc("TRN2", target_bir_lowering=False, debug=False)

    xT_a = nc.dram_tensor("xT", [D, N], bf16, kind="ExternalInput").ap()
    xTq_a = nc.dram_tensor("xTq", [D, QPC], bf16, kind="ExternalInput").ap()
    wq_a = nc.dram_tensor("wqT", [D, D], bf16, kind="ExternalInput").ap()
    wk_a = nc.dram_tensor("wkT", [D, D], bf16, kind="ExternalInput").ap()
    wv_a = nc.dram_tensor("wvT", [D, D], bf16, kind="ExternalInput").ap()
    wo_a = nc.dram_tensor("woT", [D, D], bf16, kind="ExternalInput").ap()
    bo_a = nc.dram_tensor("bo", [D], f32, kind="ExternalInput").ap()
    outT_a = nc.dram_tensor("outT", [D, QPC], f32, kind="ExternalOutput").ap()

    with tile.TileContext(nc) as tc:
        with (
            tc.tile_pool(name="w", bufs=1) as wpool,
            tc.tile_pool(name="xs", bufs=2) as xs,
            tc.tile_pool(name="kT", bufs=2) as kTp,
            tc.tile_pool(name="qT", bufs=2) as qTp,
            tc.tile_pool(name="vone", bufs=1) as vpool,
            tc.tile_pool(name="et", bufs=4) as etp,
            tc.tile_pool(name="rTn", bufs=1) as rTnp,
            tc.tile_pool(name="ot", bufs=2) as otp,
            tc.tile_pool(name="nrm", bufs=2) as nrm,
            tc.tile_pool(name="acc", bufs=4, space="PSUM") as psa,
            tc.tile_pool(name="sc", bufs=2, space="PSUM") as pss,
        ):
            def load_w(dram_ap, tag):
                w = wpool.tile([P, CD * 512], bf16, tag=tag)
                for cd in range(CD):
                    nc.sync.dma_start(
                        w[:, cd * 512:(cd + 1) * 512],
                        dram_ap[cd * P:(cd + 1) * P, :],
                    )
                return w

            wk = load_w(wk_a, "wk")
            wq = load_w(wq_a, "wq")
            wv = load_w(wv_a, "wvo")
            wo = load_w(wo_a, "wo")
            bo_t = wpool.tile([P, CD], f32, tag="bo")
            nc.sync.dma_start(bo_t[:], bo_a.rearrange("(c p) -> p c", p=P))

            vones = [None] * KC
            oaccs = {}
            rTns = [
                rTnp.tile([P, QPC], bf16, tag=f"rTn{c}", name=f"rTn{c}")
                for c in range(CD)
            ]

            def proj_group(w_t, hp, src_ap, nt, dst):
                """One 512-wide output block of a W.T @ x projection:
                4 streamed rhs tiles, 4 accumulating matmuls, 1 evacuation."""
                xts = []
                for cd in range(CD):
                    t = xs.tile([P, 512], bf16, tag=f"xk{cd}")
                    nc.sync.dma_start(
                        t[:], src_ap[cd * P:(cd + 1) * P, nt * 512:(nt + 1) * 512]
                    )
                    xts.append(t)
                ps = psa.tile([P, 512], f32, tag="acc")
                for cd in range(CD):
                    nc.tensor.matmul(
                        ps[:],
                        w_t[:, cd * 512 + hp * P: cd * 512 + (hp + 1) * P],
                        xts[cd][:],
                        start=(cd == 0),
                        stop=(cd == CD - 1),
                    )
                nc.vector.tensor_copy(dst, ps[:])

            def vproj_group(kc):
                """v projection for one 128-key chunk, written into the
                per-head [64 v | 1 ones] interleaved layout."""
                xts = []
                for cd in range(CD):
                    t = xs.tile([P, P], bf16, tag=f"xv{cd}")
                    nc.sync.dma_start(
                        t[:], xT_a[cd * P:(cd + 1) * P, kc * P:(kc + 1) * P]
                    )
                    xts.append(t)
                ps = psa.tile([P, 512], f32, tag="acc")
                for cd in range(CD):
                    nc.tensor.matmul(
                        ps[:],
                        xts[cd][:],
                        wv[:, cd * 512:(cd + 1) * 512],
                        start=(cd == 0),
                        stop=(cd == CD - 1),
                    )
                vt = vpool.tile([P, H * 65], bf16, tag=f"vone{kc}")
                v3 = vt[:].rearrange("p (h c) -> p h c", c=65)
                nc.vector.tensor_copy(
                    v3[:, :, 0:64], ps[:].rearrange("p (h c) -> p h c", c=64)
                )
                nc.vector.memset(v3[:, :, 64:65], 1.0)
                vones[kc] = vt

            def make_proj_thunks(hp):
                qt_t = qTp.tile([P, QPC], bf16, tag="qT")
                kt_t = kTp.tile([P, N], bf16, tag="kT")
                thunks = []
                for nt in range(QT):
                    thunks.append(
                        lambda nt=nt, qt_t=qt_t, hp=hp: proj_group(
                            wq, hp, xTq_a, nt, qt_t[:, nt * 512:(nt + 1) * 512]
                        )
                    )
                for nt in range(NT):
                    thunks.append(
                        lambda nt=nt, kt_t=kt_t, hp=hp: proj_group(
                            wk, hp, xT_a, nt, kt_t[:, nt * 512:(nt + 1) * 512]
                        )
                    )
                return qt_t, kt_t, thunks

            qts, kts = {}, {}
            qts[0], kts[0], th0 = make_proj_thunks(0)
            # Emit only the blocks needed to start attention: qT block 0 and
            # kT block 0; the rest of hp0's projections interleave into the
            # first kc loop (kT block g must land before kc reaches 4g).
            th0[0]()
            th0[QT]()
            hp0_qt1_proj = th0[1:QT]
            hp0_kt = th0[QT + 1:]
            pending = []

            for hp in range(HP):
                qt_t, kt_t = qts[hp], kts[hp]
                for t in pending:  # leftover projections for this head pair
                    t()
                pending = []
                h0, h1 = 2 * hp, 2 * hp + 1
                for qt in range(QT):
                    rA = psa.tile([P, 512], f32, tag="acc")
                    rB = psa.tile([P, 512], f32, tag="acc")
                    if qt == 1 and hp + 1 < HP:
                        qts[hp + 1], kts[hp + 1], pending = make_proj_thunks(hp + 1)
                    qA = qt_t[0:64, qt * 512:(qt + 1) * 512]
                    qB = qt_t[64:128, qt * 512:(qt + 1) * 512]
                    # Software-pipelined by one chunk: emit scores(kc) and its
                    # exp, then the AV matmuls for kc-1 — so the PE always has
                    # independent score work queued while ScalarE runs exp.
                    ets = {}

                    def av_pair(kc):
                        vt = vones[kc]
                        et = ets.pop(kc)
                        nc.tensor.matmul(
                            rA[0:65, :],
                            vt[:, h0 * 65:(h0 + 1) * 65],
                            et[:, 0:512],
                            start=(kc == 0), stop=(kc == KC - 1),
                        )
                        nc.tensor.matmul(
                            rB[0:65, :],
                            vt[:, h1 * 65:(h1 + 1) * 65],
                            et[:, 512:1024],
                            start=(kc == 0), stop=(kc == KC - 1),
                        )

                    for kc in range(KC):
                        if hp == 0 and qt == 0:
                            vproj_group(kc)
                            if hp0_kt and kc % 4 == 2:
                                hp0_kt.pop(0)()
                            if kc == 24:
                                for t in hp0_qt1_proj:
                                    t()
                                hp0_qt1_proj = []
                        sp = pss.tile([P, 1024], f32, tag="sc")
                        nc.tensor.matmul(
                            sp[:, 0:512],
                            kt_t[0:64, kc * P:(kc + 1) * P],
                            qA,
                            start=True, stop=True,
                            tile_position=(0, 0),
                        )
                        nc.tensor.matmul(
                            sp[:, 512:1024],
                            kt_t[64:128, kc * P:(kc + 1) * P],
                            qB,
                            start=True, stop=True,
                            tile_position=(64, 0),
                        )
                        et = etp.tile([P, 1024], bf16, tag="et")
                        nc.scalar.activation(et[:], sp[:], AF.Exp, scale=float(SCALE))
                        ets[kc] = et
                        if kc >= 1:
                            av_pair(kc - 1)
                        if pending and kc % 3 == 2:
                            pending.pop(0)()
                    av_pair(KC - 1)
                    # Evacuate both accumulators to SBUF immediately (frees the
                    # PSUM slots in ~0.7us each); the slow reciprocal/broadcast/
                    # multiply normalization then runs off the critical path.
                    rsbs = []
                    for r_ps in (rA, rB):
                        rsb = nrm.tile([65, 512], f32, tag="rsb", bufs=4)
                        nc.vector.tensor_copy(rsb[:], r_ps[0:65, :])
                        rsbs.append(rsb)
                    for rsb, poff in zip(rsbs, (0, 64)):
                        rc = nrm.tile([1, 512], f32, tag="rc")
                        nc.vector.reciprocal(rc[:], rsb[64:65, :])
                        bc = nrm.tile([64, 512], f32, tag="bc")
                        nc.gpsimd.partition_broadcast(bc[:], rc[:])
                        nc.vector.tensor_tensor(
                            rTns[hp][poff:poff + 64, qt * 512:(qt + 1) * 512],
                            rsb[0:64, :],
                            bc[:],
                            op=OP.mult,
                        )
                    # Incremental output projection: this head pair's partial
                    # contribution, accumulated in SBUF so nothing but the
                    # last pair's add remains after the attention loop.
                    for doc in range(CD):
                        ps = psa.tile([P, 512], f32, tag="acc")
                        nc.tensor.matmul(
                            ps[:],
                            wo[:, hp * 512 + doc * P: hp * 512 + (doc + 1) * P],
                            rTns[hp][:, qt * 512:(qt + 1) * 512],
                            start=True, stop=True,
                        )
                        if hp == 0:
                            oa = otp.tile([P, 512], f32, tag=f"oacc{qt}{doc}",
                                          bufs=1, name=f"oacc{qt}{doc}")
                            oaccs[(qt, doc)] = oa
                            nc.vector.tensor_copy(oa[:], ps[:])
                        else:
                            oa = oaccs[(qt, doc)]
                            nc.vector.tensor_tensor(oa[:], oa[:], ps[:], op=OP.add)
                        if hp == HP - 1:
                            ot = otp.tile([P, 512], f32, tag="ot")
                            nc.vector.tensor_tensor(
                                ot[:],
                                oa[:],
                                bo_t[:, doc:doc + 1].to_broadcast((P, 512)),
                                op=OP.add,
                            )
                            nc.sync.dma_start(
                                outT_a[doc * P:(doc + 1) * P,
                                       qt * 512:(qt + 1) * 512],
                                ot[:],
                            )

    nc.compile()
    return nc


def _get_program():
    global _PROGRAM
    if _PROGRAM is None:
        _PROGRAM = _build_program()
    return _PROGRAM


def kernel(x, Wq, Wk, Wv, Wo, bo, gamma_q, gamma_k, gamma_v, gamma_out):
    from concourse import bass_utils

    import ml_dtypes

    bf16 = ml_dtypes.bfloat16
    x = np.asarray(x, dtype=np.float32)
    f32 = np.float32
    WqT = np.ascontiguousarray((np.asarray(Wq, f32).T * np.asarray(gamma_q, f32)[None, :]).astype(bf16))
    WkT = np.ascontiguousarray((np.asarray(Wk, f32).T * np.asarray(gamma_k, f32)[None, :]).astype(bf16))
    WvT = np.ascontiguousarray((np.asarray(Wv, f32).T * np.asarray(gamma_v, f32)[None, :]).astype(bf16))
    WoT = np.ascontiguousarray((np.asarray(Wo, f32).T * np.asarray(gamma_out, f32)[None, :]).astype(bf16))
    bo_s = np.ascontiguousarray(np.asarray(gamma_out, f32) * np.asarray(bo, f32))

    xT = np.ascontiguousarray(x.transpose(0, 2, 1).astype(bf16))  # [B, D, N]

    in_maps = []
    for c in range(NCORES):
        b, q0 = c // 4, (c % 4) * QPC
        in_maps.append({
            "xT": xT[b],
            "xTq": np.ascontiguousarray(xT[b][:, q0:q0 + QPC]),
            "wqT": WqT, "wkT": WkT, "wvT": WvT, "woT": WoT,
            "bo": bo_s,
        })

    nc = _get_program()
    res = bass_utils.run_bass_kernel_spmd(nc, in_maps, core_ids=list(range(NCORES)))
    global LAST_RESULT
    LAST_RESULT = res

    out = np.empty((B, N, D), np.float32)
    for c in range(NCORES):
        b, q0 = c // 4, (c % 4) * QPC
        out[b, q0:q0 + QPC, :] = res.results[c]["outT"].T
    return out



# revision 4
# speedup vs baseline: 1.0366x; 1.0366x over previous
"""Trainium2 Bass kernel: fused multi-head self-attention block (CrossAttention).

Sharding: 8 cores = 2 batches x 4 head-pairs. Core c -> batch b = c // 4,
head pair hp = c % 4 (heads 2hp, 2hp+1 = channels hp*128 .. hp*128+127).
Each core projects Q/K/V for its own 128 channels only (no replicated
projection work), runs attention for its 2 heads over the full 4096
queries/keys of its batch, and computes the row-parallel partial output
projection outT_part[do, q] = Wo[:, hp-channels].T-block @ rTn. The host
sums the 4 per-core partials of each batch and adds the bias
(tensor-parallel to_out row split; the gather IS the reduction).

On-chip dataflow (per core, bf16 matmuls, fp32 PSUM):
  - xT [512, 4096] resident in SBUF (one DMA load, reused by Q/K/V proj)
  - kT/qT [128ch, 4096] = W-slice.T @ xT, 8 blocks each, JIT-interleaved
  - v "vones" [128k, 2*65] per key chunk: [64 v | 1 ones] per head so the
    attention rowsum falls out of the AV matmul for free
  - scores tile [128 key, 2x512 q] = two K=64 matmuls packed as PE row
    tiles (0,0)/(64,0) into one 2-bank PSUM tile
  - E = exp(SCALE * scores) on ScalarE (the rate-limiting engine:
    256 x [128,1024] activations ~ 280us), PSUM -> SBUF bf16
  - rT[65, q] += vones_h.T @ E_h accumulated over 32 key chunks in PSUM
  - normalization: reciprocal_approx_fast on the rowsum row, GpSimd
    partition broadcast, DVE multiply -> rTn bf16
  - partial out proj: 4 K=128 matmuls -> fp32 -> DMA out
"""

import os
import sys

import numpy as np

for _p in ("/opt/trn_rl_repo", "/root/.axon_site/_ro/trn_rl_repo"):
    if os.path.isdir(_p) and _p not in sys.path:
        sys.path.append(_p)

B, N, D = 2, 4096, 512
H, DH = 8, 64
SCALE = DH ** -0.5
NCORES = 8
P = 128
CD = D // P              # 4 contraction chunks of 128
KC = N // P              # 32 key chunks of 128
NT = N // 512            # 8 column blocks of 512 (key and query)

_PROGRAM = None
LAST_RESULT = None


def _build_program():
    import concourse.tile as tile
    from concourse import bacc, mybir

    f32 = mybir.dt.float32
    bf16 = mybir.dt.bfloat16
    AF = mybir.ActivationFunctionType
    OP = mybir.AluOpType

    nc = bacc.Bacc("TRN2", target_bir_lowering=False, debug=False)

    xT_a = nc.dram_tensor("xT", [D, N], bf16, kind="ExternalInput").ap()
    wqkv_a = nc.dram_tensor("wqkv", [P, 3 * CD * P], bf16, kind="ExternalInput").ap()
    wo_a = nc.dram_tensor("wo", [P, D], bf16, kind="ExternalInput").ap()
    outT_a = nc.dram_tensor("outT", [D, N], f32, kind="ExternalOutput").ap()

    with tile.TileContext(nc) as tc:
        with (
            tc.tile_pool(name="w", bufs=1) as wpool,
            tc.tile_pool(name="xt", bufs=1) as xtp,
            tc.tile_pool(name="kt", bufs=1) as ktp,
            tc.tile_pool(name="qt", bufs=1) as qtp,
            tc.tile_pool(name="vone", bufs=1) as vpool,
            tc.tile_pool(name="et", bufs=4) as etp,
            tc.tile_pool(name="rTn", bufs=1) as rTnp,
            tc.tile_pool(name="ot", bufs=2) as otp,
            tc.tile_pool(name="nrm", bufs=2) as nrm,
            tc.tile_pool(name="psa", bufs=2, space="PSUM") as psa,
            tc.tile_pool(name="sc", bufs=2, space="PSUM") as pss,
        ):
            wqkv = wpool.tile([P, 3 * CD * P], bf16, tag="wqkv")
            nc.sync.dma_start(wqkv[:], wqkv_a)
            wo = wpool.tile([P, D], bf16, tag="wo")
            nc.sync.dma_start(wo[:], wo_a)

            xt = xtp.tile([P, CD * N], bf16, tag="xt")
            # first 512 columns land first so projections can start early
            for cd in range(CD):
                nc.sync.dma_start(
                    xt[:, cd * N: cd * N + 512], xT_a[cd * P:(cd + 1) * P, 0:512]
                )
            for cd in range(CD):
                nc.sync.dma_start(
                    xt[:, cd * N + 512:(cd + 1) * N],
                    xT_a[cd * P:(cd + 1) * P, 512:N],
                )

            vones = vpool.tile([P, KC * 2 * 65], bf16, tag="vone")
            nc.vector.memset(vones[:], 1.0)  # ones columns; v cols overwritten

            kt = ktp.tile([P, N], bf16, tag="kt")
            qt = qtp.tile([P, N], bf16, tag="qt")
            rTn = rTnp.tile([P, N], bf16, tag="rTn")

            def kq_proj(w_idx, nt, dst):
                """One 512-wide block of kT/qT = W-slice.T @ xT."""
                ps = psa.tile([P, 512], f32, tag="prj")
                for cd in range(CD):
                    nc.tensor.matmul(
                        ps[:],
                        wqkv[:, (w_idx * CD + cd) * P:(w_idx * CD + cd + 1) * P],
                        xt[:, cd * N + nt * 512: cd * N + (nt + 1) * 512],
                        start=(cd == 0),
                        stop=(cd == CD - 1),
                    )
                nc.vector.tensor_copy(dst[:, nt * 512:(nt + 1) * 512], ps[:])

            def vproj(kc):
                """V projection for one 128-key chunk into the vones layout."""
                ps = psa.tile([P, P], f32, tag="prj")
                for cd in range(CD):
                    nc.tensor.matmul(
                        ps[:],
                        xt[:, cd * N + kc * P: cd * N + (kc + 1) * P],
                        wqkv[:, (2 * CD + cd) * P:(2 * CD + cd + 1) * P],
                        start=(cd == 0),
                        stop=(cd == CD - 1),
                    )
                v3 = vones[:, kc * 130:(kc + 1) * 130].rearrange(
                    "p (h c) -> p h c", c=65
                )
                nc.vector.tensor_copy(
                    v3[:, :, 0:64], ps[:].rearrange("p (h c) -> p h c", c=64)
                )

            kq_proj(1, 0, kt)
            kq_proj(0, 0, qt)

            def oproj(qts):
                """Partial output projection for one 512-query block."""
                for doc in range(CD):
                    ps = psa.tile([P, 512], f32, tag="prj")
                    nc.tensor.matmul(
                        ps[:],
                        wo[:, doc * P:(doc + 1) * P],
                        rTn[:, qts * 512:(qts + 1) * 512],
                        start=True, stop=True,
                    )
                    ot = otp.tile([P, 512], f32, tag="ot")
                    nc.vector.tensor_copy(ot[:], ps[:])
                    nc.sync.dma_start(
                        outT_a[doc * P:(doc + 1) * P, qts * 512:(qts + 1) * 512],
                        ot[:],
                    )

            for qts in range(NT):
                rA = psa.tile([P, 512], f32, tag="rAB")
                rB = psa.tile([P, 512], f32, tag="rAB")
                qA = qt[0:64, qts * 512:(qts + 1) * 512]
                qB = qt[64:128, qts * 512:(qts + 1) * 512]
                ets = {}

                def av_pair(kc, rA=rA, rB=rB):
                    et = ets.pop(kc)
                    nc.tensor.matmul(
                        rA[0:65, :],
                        vones[:, kc * 130: kc * 130 + 65],
                        et[:, 0:512],
                        start=(kc == 0), stop=(kc == KC - 1),
                    )
                    nc.tensor.matmul(
                        rB[0:65, :],
                        vones[:, kc * 130 + 65:(kc + 1) * 130],
                        et[:, 512:1024],
                        start=(kc == 0), stop=(kc == KC - 1),
                    )

                for kc in range(KC):
                    if qts == 0:
                        vproj(kc)
                        g = kc // 4 + 1
                        if kc % 4 == 1 and g < NT:
                            kq_proj(1, g, kt)
                        if kc % 4 == 3 and g < NT:
                            kq_proj(0, g, qt)
                    sp = pss.tile([P, 1024], f32, tag="sc")
                    nc.tensor.matmul(
                        sp[:, 0:512],
                        kt[0:64, kc * P:(kc + 1) * P],
                        qA,
                        start=True, stop=True,
                        tile_position=(0, 0),
                    )
                    nc.tensor.matmul(
                        sp[:, 512:1024],
                        kt[64:128, kc * P:(kc + 1) * P],
                        qB,
                        start=True, stop=True,
                        tile_position=(64, 0),
                    )
                    et = etp.tile([P, 1024], bf16, tag="et")
                    nc.scalar.activation(et[:], sp[:], AF.Exp, scale=float(SCALE))
                    ets[kc] = et
                    if kc >= 1:
                        av_pair(kc - 1)
                    if kc == 1 and qts >= 2:
                        oproj(qts - 2)
                av_pair(KC - 1)

                # normalization: evacuate PSUM fast, then recip/broadcast/mult
                # off the critical path; rTn feeds oproj two qts later.
                for r_ps, poff in ((rA, 0), (rB, 64)):
                    rsb = nrm.tile([65, 512], f32, tag="rsb", bufs=4)
                    nc.vector.tensor_copy(rsb[:], r_ps[0:65, :])
                    rc = nrm.tile([1, 512], f32, tag="rc")
                    nc.vector.reciprocal(rc[:], rsb[64:65, :])
                    bc = nrm.tile([64, 512], f32, tag="bc")
                    nc.gpsimd.partition_broadcast(bc[:], rc[:])
                    nc.vector.tensor_tensor(
                        rTn[poff:poff + 64, qts * 512:(qts + 1) * 512],
                        rsb[0:64, :],
                        bc[:],
                        op=OP.mult,
                    )
            oproj(NT - 2)
            oproj(NT - 1)

    nc.compile()
    return nc


def _get_program():
    global _PROGRAM
    if _PROGRAM is None:
        _PROGRAM = _build_program()
    return _PROGRAM


def _pack_w(wT_slice):
    """[512, 128] weight slice -> [128, CD*128] tile layout (cd-major)."""
    return np.ascontiguousarray(
        wT_slice.reshape(CD, P, P).transpose(1, 0, 2).reshape(P, CD * P)
    )


def kernel(x, Wq, Wk, Wv, Wo, bo, gamma_q, gamma_k, gamma_v, gamma_out):
    from concourse import bass_utils

    import ml_dtypes

    bf16 = ml_dtypes.bfloat16
    f32 = np.float32
    x = np.asarray(x, dtype=f32)
    WqT = np.asarray(Wq, f32).T * np.asarray(gamma_q, f32)[None, :]
    WkT = np.asarray(Wk, f32).T * np.asarray(gamma_k, f32)[None, :]
    WvT = np.asarray(Wv, f32).T * np.asarray(gamma_v, f32)[None, :]
    WoT = np.asarray(Wo, f32).T * np.asarray(gamma_out, f32)[None, :]
    bo_s = np.asarray(gamma_out, f32) * np.asarray(bo, f32)

    xT = np.ascontiguousarray(x.transpose(0, 2, 1).astype(bf16))  # [B, D, N]

    in_maps = []
    for c in range(NCORES):
        b, hp = c // 4, c % 4
        cols = slice(hp * P, (hp + 1) * P)
        wqkv = np.concatenate(
            [
                _pack_w(WqT[:, cols].astype(bf16)),
                _pack_w(WkT[:, cols].astype(bf16)),
                _pack_w(WvT[:, cols].astype(bf16)),
            ],
            axis=1,
        )
        in_maps.append({
            "xT": xT[b],
            "wqkv": np.ascontiguousarray(wqkv),
            "wo": np.ascontiguousarray(WoT[cols.start:cols.stop, :].astype(bf16)),
        })

    nc = _get_program()
    res = bass_utils.run_bass_kernel_spmd(nc, in_maps, core_ids=list(range(NCORES)))
    global LAST_RESULT
    LAST_RESULT = res

    out = np.empty((B, N, D), np.float32)
    for b in range(B):
        acc = res.results[4 * b]["outT"].astype(f32)
        for hp in range(1, 4):
            acc = acc + res.results[4 * b + hp]["outT"]
        out[b] = acc.T + bo_s[None, :]
    return out
